# revision 1
# baseline (speedup 1.0000x reference)
"""KoLeo loss kernel for Trainium2 (8 NeuronCores).

Computes -mean(log(||x_i - x_{nn(i)} + eps||)) where x = row-normalized
student_output and nn(i) is the nearest neighbor by max inner product
(diagonal excluded).

Strategy: for unit vectors, ||x_i - x_j||^2 = 2 - 2*<x_i, x_j>, so only the
per-row max off-diagonal inner product m_i is needed. Each core handles a
2048-row block: it receives the full matrix rotated so its own rows sit at
local rows 0..2047 (making the dots diagonal position identical on every
core - SPMD-uniform masking), normalizes + transposes the matrix on-chip,
computes its [2048, 16384] block of inner products with float32r matmuls,
and reduces to per-row maxes. The final log-mean runs on host from the 8
tiny [128,16] outputs.
"""

import numpy as np

import concourse.bass as bass
import concourse.mybir as mybir
import concourse.tile as tile
from concourse import bacc
from concourse import bass_utils
from concourse.masks import make_identity

N = 16384
D = 256
NCORES = 8
ROWS = N // NCORES          # 2048 rows per core
ITILES = ROWS // 128        # 16 i-tiles per core
NT = N // 128               # 128 row-tiles of the full matrix
GW = 2048                   # j-group width (4 PSUM banks of fp32)
NGROUPS = N // GW           # 8 j-groups
NB = 16                     # row-tiles per normalization batch
EPS = 1e-8

_CACHE = {}


def _build():
    f32 = mybir.dt.float32
    f32r = mybir.dt.float32r
    AF = mybir.ActivationFunctionType
    ALU = mybir.AluOpType

    nc = bacc.Bacc("TRN2", target_bir_lowering=False, debug=False)
    x = nc.dram_tensor("x", [N, D], f32, kind="ExternalInput").ap()
    m_out = nc.dram_tensor("m_out", [128, ITILES], f32, kind="ExternalOutput").ap()

    with tile.TileContext(nc) as tc:
        with (
            tc.tile_pool(name="singles", bufs=1) as singles,
            tc.tile_pool(name="s_stage", bufs=2 * NB + 2) as s_stage,
            tc.tile_pool(name="small", bufs=6) as small,
            tc.tile_pool(name="xs", bufs=8) as xs_pool,
            tc.tile_pool(name="xt", bufs=1) as xt_pool,
            tc.tile_pool(name="scr", bufs=3) as scr_pool,
        ):
            ident = singles.tile([128, 128], f32, tag="ident")
            make_identity(nc, ident[:])

            # Diagonal knock-out mask: -3 on the diagonal of a 128x128 block.
            mneg = singles.tile([128, 128], f32, tag="mneg")
            nc.gpsimd.memset(mneg[:], 0.0)
            nc.gpsimd.affine_select(
                out=mneg[:],
                in_=mneg[:],
                compare_op=ALU.not_equal,
                fill=-3.0,
                base=0,
                pattern=[[-1, 128]],
                channel_multiplier=1,
            )

            # sum of squares per row, laid out [128, row-tile]
            ss = singles.tile([128, NT], f32, tag="ss")
            # per-row max accumulator, [128, i-tile]
            m_sb = singles.tile([128, ITILES], f32, tag="m_sb")

            # XT[k][g]: transposed normalized matrix, d-half k, j-group g.
            xt = [
                [
                    xt_pool.tile([128, GW], f32r, tag=f"xt{k}_{g}", name=f"xt{k}_{g}")
                    for g in range(NGROUPS)
                ]
                for k in range(2)
            ]

            # ---- Phase 1: normalize rows and build XT ----
            with tc.tile_pool(name="tpsum", bufs=8, space="PSUM") as tpsum:
                for b in range(NT // NB):
                    tiles = range(b * NB, (b + 1) * NB)
                    s_tiles = {}
                    for t in tiles:
                        s = s_stage.tile([128, D], f32, tag="s")
                        nc.sync.dma_start(out=s[:], in_=x[t * 128:(t + 1) * 128, :])
                        sq = small.tile([128, D], f32, tag="sq")
                        nc.scalar.activation(
                            sq[:], s[:], AF.Square, accum_out=ss[:, t:t + 1]
                        )
                        s_tiles[t] = s

                    # batched r = rsqrt(ss) with two Newton steps
                    # (ACT Sqrt is low-precision; DVE reciprocal is accurate)
                    col = (b * NB, (b + 1) * NB)
                    ssb = ss[:, col[0]:col[1]]
                    sq_b = small.tile([128, NB], f32, tag="sqb")
                    nc.scalar.activation(sq_b[:], ssb, AF.Sqrt)
                    r = small.tile([128, NB], f32, tag="r")
                    nc.vector.reciprocal(r[:], sq_b[:])
                    for _ in range(2):
                        t1 = small.tile([128, NB], f32, tag="t1")
                        nc.vector.tensor_mul(t1[:], r[:], r[:])
                        nc.vector.tensor_mul(t1[:], t1[:], ssb)
                        # t1 <- 1.5 - 0.5*t1
                        nc.scalar.activation(t1[:], t1[:], AF.Copy, scale=-0.5, bias=1.5)
                        r2 = small.tile([128, NB], f32, tag="r")
                        nc.vector.tensor_mul(r2[:], r[:], t1[:])
                        r = r2

                    for t in tiles:
                        w = t - b * NB
                        xs = xs_pool.tile([128, D], f32, tag="xs")
                        nc.vector.tensor_scalar_mul(
                            xs[:], s_tiles[t][:], r[:, w:w + 1]
                        )
                        g, pos = t // 16, t % 16
                        for k in range(2):
                            pt = tpsum.tile([128, 128], f32, tag="pt")
                            nc.tensor.transpose(
                                pt[:], xs[:, k * 128:(k + 1) * 128], ident[:]
                            )
                            nc.any.tensor_copy(
                                xt[k][g][:, pos * 128:(pos + 1) * 128], pt[:]
                            )

            # ---- Phase 2: dots + row max ----
            with tc.tile_pool(name="dpsum", bufs=2, space="PSUM") as dpsum:
                for t in range(ITILES):
                    lhs = [xt[k][0][:, t * 128:(t + 1) * 128] for k in range(2)]
                    mp = small.tile([128, NGROUPS + 2], f32, tag="mp")
                    nc.vector.memset(mp[:], -3.0)
                    for g in range(NGROUPS):
                        pg = dpsum.tile([128, GW], f32, tag="pg")
                        for s4 in range(GW // 512):
                            o = pg[:, s4 * 512:(s4 + 1) * 512]
                            j0 = s4 * 512
                            nc.tensor.matmul(
                                o, lhs[0], xt[0][g][:, j0:j0 + 512],
                                start=True, stop=False,
                            )
                            nc.tensor.matmul(
                                o, lhs[1], xt[1][g][:, j0:j0 + 512],
                                start=False, stop=True,
                            )
                        if g == 0:
                            # group 0 holds the diagonal at column 128t+p.
                            # Mask only the 128-wide block, reduce around it.
                            db = 128 * t
                            nc.vector.tensor_add(
                                pg[:, db:db + 128], pg[:, db:db + 128], mneg[:]
                            )
                            nc.vector.reduce_max(
                                mp[:, 0:1], pg[:, db:db + 128],
                                axis=mybir.AxisListType.X,
                            )
                            if t > 0:
                                nc.vector.reduce_max(
                                    mp[:, 8:9], pg[:, 0:db],
                                    axis=mybir.AxisListType.X,
                                )
                            if t < ITILES - 1:
                                nc.vector.reduce_max(
                                    mp[:, 9:10], pg[:, db + 128:GW],
                                    axis=mybir.AxisListType.X,
                                )
                        else:
                            nc.vector.reduce_max(
                                mp[:, g:g + 1], pg[:], axis=mybir.AxisListType.X
                            )
                    nc.vector.reduce_max(
                        m_sb[:, t:t + 1], mp[:], axis=mybir.AxisListType.X
                    )

            nc.sync.dma_start(out=m_out, in_=m_sb[:])

    nc.compile()
    return nc


def _get_nc():
    if "nc" not in _CACHE:
        _CACHE["nc"] = _build()
    return _CACHE["nc"]


def kernel(student_output: np.ndarray) -> np.ndarray:
    s = np.ascontiguousarray(np.asarray(student_output, dtype=np.float32))
    assert s.shape == (N, D)

    nc = _get_nc()
    in_maps = [
        {"x": np.ascontiguousarray(np.roll(s, -c * ROWS, axis=0))}
        for c in range(NCORES)
    ]
    import os
    kwargs = {}
    if os.environ.get("KOLEO_TRACE"):
        kwargs = {"trace": True, "tmpdir": os.environ.get("KOLEO_TRACE_DIR") or None}
    res = bass_utils.run_bass_kernel_spmd(
        nc, in_maps, core_ids=list(range(NCORES)), **kwargs
    )
    _CACHE["last_results"] = res

    m = np.concatenate(
        [res.results[c]["m_out"].T.reshape(ROWS) for c in range(NCORES)]
    )  # [N] per-row max inner product, global row order

    d2 = np.maximum(2.0 - 2.0 * m.astype(np.float64), 0.0)
    loss = -np.mean(np.log(np.sqrt(d2) + EPS))
    return np.array(loss, dtype=np.float32)

